# revision 18
# baseline (speedup 1.0000x reference)
"""Trainium2 Bass kernel for CNF probability-flow ODE sampling.

Problem: integrate the VP probability-flow ODE for 32768 independent samples
(dim 16) from t=1 down to t=1e-5; reproduce the reference's 100-step Tsit5
solution within the harness' 2e-2 relative-error gate.  Each drift eval runs
a 4-layer MLP (81 -> 512 -> 512 -> 512 -> 16, gelu-tanh activations).

Numerical scheme (equivalent to the reference within ~2e-3, measured on the
harness inputs):
  - DOPRI5 (FSAL) instead of Tsit5: same 5th order, but ||b||_2 ~= 0.87 vs
    Tsit5's 4.29, so per-eval matmul rounding noise accumulates ~5x more
    slowly across steps.  FSAL costs one extra drift eval per sample group.
  - N_STEPS graded steps on t_k = T0 + (T1-T0)*(1-k/N)^P with P=0.67: the
    truncation error concentrates near t=1 (large beta), so steps are packed
    there.  Measured truncation vs the 100-step reference (fp32 CPU):
    N=4 -> 2.0e-3, N=5 -> 5.3e-4.  fp32r matmul noise adds ~6e-4.
  - Everything fp32/fp32r.  fp8 (e4m3) matmuls were measured to DIVERGE
    (the quantized drift field destabilizes the flow) - do not revisit.

Kernel strategy (data-parallel over samples, 8 cores x 4096 samples):
  - All state + weights live in SBUF for the whole integration.
  - Activations stored feature-major: h^T [512 feat (partitions x4 chunks),
    512 samples (free)], so matmuls are plain lhsT.T @ rhs with K on
    partitions and samples on the moving free dim (N=512).
  - float32r matmuls (full fp32 data, 1 cycle/row at N=512).
  - The conditioning input x, b1 and the time feature are folded into the
    L1 lhsT bias row; L1 is K=32 (16 theta rows + 1 bias row vs ones + 15
    zero pad).  The bias row is affine in the eval time, so the active L1
    lhsT is one DVE op with a float-immediate time scalar.
  - The whole integration is unrolled (no hardware loop): with a graded
    grid every RK combination scalar is a compile-time float immediate.
  - 2 sequential groups of 4 sample tiles; tiles are layer-interleaved so
    PE always has an independent matmul group while ACT runs a gelu.
"""

import numpy as np

import concourse.bass as bass
import concourse.mybir as mybir
import concourse.tile as tile
from concourse.bass_utils import run_bass_kernel_spmd

F32 = mybir.dt.float32
F32R = mybir.dt.float32r
ALU = mybir.AluOpType
ACTF = mybir.ActivationFunctionType

N_CORES = 8
DIM_P, DIM_D, HID = 16, 64, 512
N_SAMPLES = 32768
PER_CORE = N_SAMPLES // N_CORES      # 4096
NT = 512                             # samples per tile (matmul moving dim)
T1, T0 = 1.0, 1e-05
BETA_MIN, BETA_MAX = 0.1, 20.0
BD = BETA_MAX - BETA_MIN

# Graded time grid: finer near t=1 where beta (and the truncation error) is
# largest.  4 DOPRI5 steps on this grid reproduce the reference 100-step
# Tsit5 solution to ~2e-3 relative error on the harness inputs.
N_STEPS = 4
GRID_P = 0.67
GRID_T = [float(T0 + (T1 - T0) * (1.0 - k / N_STEPS) ** GRID_P)
          for k in range(N_STEPS + 1)]

# DOPRI5 tableau (FSAL): k1 of each step is the previous step's trailing
# drift eval at (t+dt, y_new).
C = [0.0, 0.2, 0.3, 0.8, 8.0 / 9.0, 1.0]   # C_j for j=1..6 (C[0]=stage1)
A = {
    2: [0.2],
    3: [3.0 / 40.0, 9.0 / 40.0],
    4: [44.0 / 45.0, -56.0 / 15.0, 32.0 / 9.0],
    5: [19372.0 / 6561.0, -25360.0 / 2187.0, 64448.0 / 6561.0,
        -212.0 / 729.0],
    6: [9017.0 / 3168.0, -355.0 / 33.0, 46732.0 / 5247.0, 49.0 / 176.0,
        -5103.0 / 18656.0],
}
B = [35.0 / 384.0, 0.0, 500.0 / 1113.0, 125.0 / 192.0, -2187.0 / 6784.0,
     11.0 / 84.0]


def _beta_factor(t_eval):
    """k_j = beta_factor(t_j) * q_j with q_j = theta_j + score_j."""
    return -0.5 * (BETA_MIN + t_eval * BD)


def prepare_host_inputs(x, init_theta, W1, b1, W2, b2, W3, b3, Wout, bout,
                        parameter_mean, parameter_std, data_mean, data_std):
    """Fold x / b1 / time feature into packed weight tensors (numpy, host)."""
    x = np.asarray(x, np.float32)
    x_n = (x - np.asarray(data_mean, np.float32)) / np.asarray(data_std, np.float32)
    W1 = np.asarray(W1, np.float32)
    w1_theta = W1[0:DIM_P, :]                    # [16, 512]
    w1_x = W1[DIM_P:DIM_P + DIM_D, :]            # [64, 512]
    w1_t = W1[DIM_P + DIM_D, :]                  # [512]
    base_const = x_n @ w1_x + np.asarray(b1, np.float32)   # [512]

    # w1pack column blocks of 512:
    #   block 0: w1tpad (row 16 = w1_t, rest 0)
    #   block 1: rows 0:16 = W1_theta, row 16 = base_const, rest 0
    #   block 2: "onespad" (row 16 = 1, rest 0) - static rows for stage tiles
    # The active L1 lhsT for an eval at time t is block0 * t + block1.
    w1pack = np.zeros((32, 3 * HID), np.float32)
    w1pack[16, 0:HID] = w1_t
    w1pack[0:DIM_P, HID:2 * HID] = w1_theta
    w1pack[16, HID:2 * HID] = base_const
    w1pack[16, 2 * HID:3 * HID] = 1.0

    w2pack = np.ascontiguousarray(
        np.asarray(W2, np.float32).reshape(4, 128, HID).transpose(1, 0, 2)
    ).reshape(128, 4 * HID)
    w3pack = np.ascontiguousarray(
        np.asarray(W3, np.float32).reshape(4, 128, HID).transpose(1, 0, 2)
    ).reshape(128, 4 * HID)
    wopack = np.ascontiguousarray(
        np.asarray(Wout, np.float32).reshape(4, 128, DIM_P).transpose(1, 0, 2)
    ).reshape(128, 4 * DIM_P)

    # smallconsts columns: 0 bout, 1 pmean, 2 pstd
    smallconsts = np.zeros((DIM_P, 8), np.float32)
    smallconsts[:, 0] = np.asarray(bout, np.float32)
    smallconsts[:, 1] = np.asarray(parameter_mean, np.float32)
    smallconsts[:, 2] = np.asarray(parameter_std, np.float32)

    return {
        "w1pack": w1pack, "w2pack": w2pack, "w3pack": w3pack,
        "wopack": wopack, "smallconsts": smallconsts,
        "b2": np.asarray(b2, np.float32), "b3": np.asarray(b3, np.float32),
        "theta": np.ascontiguousarray(np.asarray(init_theta, np.float32)),
    }


# megapack column layout (fp32 elements per partition, 128 partitions):
#   [0 : 2048)            w2pack           (rows 0:128)
#   [2048 : 4096)         w3pack           (rows 0:128)
#   [4096 : 4160)         wopack           (rows 0:128)
#   [4160 : 4168)         smallconsts      (rows 0:16)
#   [4168 : 5704)         w1pack (3*512)   (rows 0:32)
#   [5704 : 5704+ntiles*512)  thetapack    (rows 0:32)
MEGA_W2, MEGA_W3, MEGA_WO, MEGA_SC, MEGA_W1, MEGA_TH = (
    0, 2048, 4096, 4160, 4168, 5704)


def pack_mega(host, theta_slice):
    n = theta_slice.shape[0]
    ntiles = n // NT
    cols = MEGA_TH + ntiles * NT
    mega = np.zeros((128, cols), np.float32)
    mega[:, MEGA_W2:MEGA_W2 + 4 * HID] = host["w2pack"]
    mega[:, MEGA_W3:MEGA_W3 + 4 * HID] = host["w3pack"]
    mega[:, MEGA_WO:MEGA_WO + 4 * DIM_P] = host["wopack"]
    mega[0:DIM_P, MEGA_SC:MEGA_SC + 8] = host["smallconsts"]
    mega[0:32, MEGA_W1:MEGA_W1 + 3 * HID] = host["w1pack"]
    mega[0:32, MEGA_TH:] = pack_theta(theta_slice).reshape(
        ntiles, 32, NT).transpose(1, 0, 2).reshape(32, ntiles * NT)
    return mega


def pack_theta(theta_slice):
    """[n, 16] -> [ntiles*32, NT]: per tile rows 0:16 = theta^T, row 16 = 1."""
    n = theta_slice.shape[0]
    assert n % NT == 0
    ntiles = n // NT
    out = np.zeros((ntiles * 32, NT), np.float32)
    for t in range(ntiles):
        out[t * 32:t * 32 + DIM_P, :] = theta_slice[t * NT:(t + 1) * NT].T
        out[t * 32 + 16, :] = 1.0
    return out


def _fix_sync_wait_overflow(nc, wsem):
    """Walrus enforces small per-instruction sync-wait limits (1 for
    Matmult/CTRL-type instructions).  Tile can emit more.  Two safe local
    rewrites: (a) drop same-engine waits (engines execute and complete their
    own instructions strictly in program order, so they are implicit); (b)
    if more than one cross-engine wait remains, keep one and move the rest
    onto prepended same-engine NoOps (the sequencer processes them in order,
    so all waits are still satisfied before the instruction issues)."""
    import bass_rust

    def waits_of(inst):
        si = inst.sync_info
        return list(si.on_wait) if si else []

    def upds_of(inst):
        si = inst.sync_info
        return list(si.on_update) if si else []

    def set_sync(inst, waits, upds):
        inst.sync_info = bass_rust.SyncInfo(on_wait=waits, on_update=upds)

    def base_eng(w):
        return w.ant_name.split("_")[0]

    self_eng = {
        mybir.InstMatmult: "PE",
        mybir.InstActivation: "Activation",
        mybir.InstTensorScalarPtr: "DVE",
        mybir.InstTensorTensor: "DVE",
        mybir.InstTensorCopy: "DVE",
        mybir.InstMemset: "DVE",
    }

    nsplit = [0]
    fn = nc.m.functions[0]
    for blk in fn.blocks:
        idx = 0
        while idx < len(blk.instructions):
            inst = blk.instructions[idx]
            waits = waits_of(inst)
            eng = self_eng.get(type(inst))
            if eng is not None and len(waits) > 1:
                kept = [w for w in waits if base_eng(w) != eng]
                if len(kept) > 1:
                    for w in kept[1:]:
                        nsplit[0] += 1
                        nop = mybir.InstNoOp(
                            name=f"{inst.name}-wsplit{nsplit[0]}",
                            engine=inst.engine,
                            bass_nofuse=True,
                            sync_info=bass_rust.SyncInfo(
                                on_wait=[w],
                                on_update=[bass_rust.SyncUpdate(
                                    sync_type="semaphore", id=wsem.num,
                                    update_mode="sem-inc",
                                    ant_name=wsem.name, update_value=1)]),
                        )
                        blk.instructions.insert(idx, nop)
                        idx += 1
                    kept = kept[:1]
                set_sync(inst, kept, upds_of(inst))
            elif isinstance(inst, mybir.InstDrain) and len(waits) > 1:
                # Drains take a single wait.  Engine-tick waits on a drain are
                # redundant: every drain here is followed by the all-engine
                # barrier whose per-engine drains flush each engine's own
                # pipeline.  DMA-queue waits are NOT covered by engine drains
                # and must stay.
                kept = [w for w in waits if base_eng(w) not in
                        ("PE", "Activation", "DVE", "Pool", "SP")]
                if not kept:
                    kept = [w for w in waits if base_eng(w) == "DVE"]
                assert len(kept) <= 1, (blk.name, inst.name, waits)
                set_sync(inst, kept, upds_of(inst))
            idx += 1


def build_program(grid=None, per_core=PER_CORE, tiles_per_group=4,
                  with_b23=False):
    """Build the Bass/Tile program (single SPMD program, run on 8 cores).

    Three sequential TileContexts: (1) weight/const load + fp32r rounding,
    (2) the fully-unrolled integration (no DMA at all inside), (3) output
    stores.  `grid` is the list of time points (T1 -> T0); every RK scalar
    derived from it is emitted as a float immediate.
    """
    if grid is None:
        grid = GRID_T
    n_steps = len(grid) - 1
    assert per_core % (NT * tiles_per_group) == 0
    n_groups = per_core // (NT * tiles_per_group)
    n_tiles = per_core // NT
    TPG = tiles_per_group

    nc = bass.Bass("TRN2", target_bir_lowering=False, debug=False)

    # Allocated up front so no TileContext reuses its hardware slot: the
    # post-pass wait-splitting NoOps tick this sem (CoreSim requires every
    # engine instruction to carry an on_update); nobody waits on it.
    wsem = nc.alloc_semaphore("wsplit")

    mega_cols = MEGA_TH + n_tiles * NT
    mega_d = nc.dram_tensor("megapack", [128, mega_cols], F32,
                            kind="ExternalInput").ap()
    if with_b23:
        b23_d = nc.dram_tensor("b23pack", [128, 8], F32, kind="ExternalInput").ap()
    out_d = nc.dram_tensor("out", [n_tiles * DIM_P, NT], F32,
                           kind="ExternalOutput").ap()

    GELU = ACTF.Gelu_apprx_tanh

    def sb(name, shape, dtype):
        return nc.alloc_sbuf_tensor(name, list(shape), dtype).ap()

    # persistent SBUF tensors (outside any tile pool; survive across contexts)
    mega_sb = sb("mega", [128, mega_cols], F32)
    w1c_sb = mega_sb[0:32, MEGA_W1:MEGA_W1 + 3 * HID]
    ypack_sb = mega_sb[0:32, MEGA_TH:MEGA_TH + n_tiles * NT]
    bout_ap = mega_sb[0:DIM_P, MEGA_SC + 0:MEGA_SC + 1]
    pmean_ap = mega_sb[0:DIM_P, MEGA_SC + 1:MEGA_SC + 2]
    pstd_ap = mega_sb[0:DIM_P, MEGA_SC + 2:MEGA_SC + 3]
    pad_sb = w1c_sb[:, 2 * HID:3 * HID]

    # one active-L1 buffer per stage slot 1..6 (FSAL reuses slot 6, whose
    # eval time t+C6*dt == t+dt): all six are computed up front per step,
    # off the critical path.
    w1act_sb = {s: sb(f"w1act{s}", [32, HID], F32R) for s in (1, 2, 3, 4, 5, 6)}
    w2_sb = sb("w2r", [128, 4 * HID], F32R)
    w3_sb = sb("w3r", [128, 4 * HID], F32R)
    wo_sb = sb("wor", [128, 4 * DIM_P], F32R)
    if with_b23:
        b23_sb = sb("b23_sb", [128, 8], F32)
    y_sb = [ypack_sb[:, gt * NT:(gt + 1) * NT] for gt in range(n_tiles)]
    yr_sb = [sb(f"yr{i}", [32, NT], F32R) for i in range(TPG)]
    th_sb = {s: [sb(f"th{s}_{i}", [32, NT], F32R) for i in range(TPG)]
             for s in (2, 3, 4, 5, 6)}
    q_sb = {j: [sb(f"q{j}_{i}", [DIM_P, NT], F32) for i in range(TPG)]
            for j in (1, 2, 3, 4, 5, 6)}

    # ---- context 1a: the single input DMA ----
    with tile.TileContext(nc):
        nc.sync.dma_start(out=mega_sb, in_=mega_d)
        if with_b23:
            nc.sync.dma_start(out=b23_sb, in_=b23_d)

    # ---- context 1b: fp32r rounding + static inits (DVE only) ----
    with tile.TileContext(nc):
        nc.vector.tensor_copy(w2_sb[:, :], mega_sb[:, MEGA_W2:MEGA_W2 + 4 * HID])
        nc.vector.tensor_copy(w3_sb[:, :], mega_sb[:, MEGA_W3:MEGA_W3 + 4 * HID])
        nc.vector.tensor_copy(wo_sb[:, :],
                              mega_sb[:, MEGA_WO:MEGA_WO + 4 * DIM_P])
        for i in range(TPG):
            nc.vector.tensor_copy(yr_sb[i][:, :], pad_sb)
            for s in (2, 3, 4, 5, 6):
                nc.vector.tensor_copy(th_sb[s][i][:, :], pad_sb)

    # ---- context 2: the integration (no DMA inside) ----
    with tile.TileContext(nc) as tc:
        from contextlib import ExitStack
        with ExitStack() as ctx:
            hs_pool = ctx.enter_context(tc.tile_pool(name="hs", bufs=4))
            hp_pool = ctx.enter_context(
                tc.tile_pool(name="hp", bufs=4, space="PSUM"))

            def set_w1act(slot, t_eval):
                # active L1 lhsT = w1tpad * t_eval + [W1_theta; base_const]
                nc.vector.scalar_tensor_tensor(
                    out=w1act_sb[slot][:, :],
                    in0=w1c_sb[:, 0:HID],
                    scalar=float(t_eval),
                    in1=w1c_sb[:, HID:2 * HID],
                    op0=ALU.mult, op1=ALU.add)

            def mlp_stage_all(s, g, slot, after_q=None):
                """One drift evaluation for all tile slots at stage s,
                emitted layer-interleaved across tiles so the scheduler's
                trace-order priorities alternate tiles (PE always has an
                independent matmul group ready while ACT runs a gelu).
                s==1 evaluates at the current y (rhs yr_sb) and is used both
                for the pre-loop k1 and the FSAL trailing eval."""
                w1a = w1act_sb[slot]
                rhs1 = [yr_sb[i] if s == 1 else th_sb[s][i] for i in range(TPG)]
                hp12 = []
                for i in range(TPG):
                    hp1 = hp_pool.tile([128, 2 * NT], F32, tag="hp", name="hp")
                    hp2 = hp_pool.tile([128, 2 * NT], F32, tag="hp", name="hp")
                    for mc in range(4):
                        pt = hp1 if mc < 2 else hp2
                        nc.tensor.matmul(
                            pt[:, (mc % 2) * NT:(mc % 2 + 1) * NT],
                            w1a[0:32, mc * 128:(mc + 1) * 128],
                            rhs1[i][0:32, :],
                            start=True, stop=True)
                    hp12.append((hp1, hp2))
                hs1 = []
                for i in range(TPG):
                    h = hs_pool.tile([128, 4 * NT], F32R, tag="hs", name="hs")
                    nc.scalar.activation(h[:, 0:2 * NT], hp12[i][0], GELU)
                    nc.scalar.activation(h[:, 2 * NT:4 * NT], hp12[i][1], GELU)
                    hs1.append(h)

                def dense_layer(w_sb, hs_in, bias_off=None):
                    hps = []
                    for i in range(TPG):
                        hp1 = hp_pool.tile([128, 2 * NT], F32, tag="hp", name="hp")
                        hp2 = hp_pool.tile([128, 2 * NT], F32, tag="hp", name="hp")
                        for mc in range(4):
                            pt = hp1 if mc < 2 else hp2
                            for kc in range(4):
                                nc.tensor.matmul(
                                    pt[:, (mc % 2) * NT:(mc % 2 + 1) * NT],
                                    w_sb[:, kc * HID + mc * 128:kc * HID + (mc + 1) * 128],
                                    hs_in[i][:, kc * NT:(kc + 1) * NT],
                                    start=(kc == 0), stop=(kc == 3))
                        if with_b23 and bias_off is not None:
                            for mc in range(4):
                                pt = hp1 if mc < 2 else hp2
                                nc.vector.tensor_scalar_add(
                                    pt[:, (mc % 2) * NT:(mc % 2 + 1) * NT],
                                    pt[:, (mc % 2) * NT:(mc % 2 + 1) * NT],
                                    b23_sb[:, bias_off + mc:bias_off + mc + 1])
                        hps.append((hp1, hp2))
                    outs = []
                    for i in range(TPG):
                        h = hs_pool.tile([128, 4 * NT], F32R, tag="hs", name="hs")
                        nc.scalar.activation(h[:, 0:2 * NT], hps[i][0], GELU)
                        nc.scalar.activation(h[:, 2 * NT:4 * NT], hps[i][1], GELU)
                        outs.append(h)
                    return outs

                hs2 = dense_layer(w2_sb, hs1, bias_off=0)
                hs3 = dense_layer(w3_sb, hs2, bias_off=4)

                # ---- Lout -> score [16, 512] per tile, then q = score+bout+in
                for i in range(TPG):
                    spt = hp_pool.tile([128, 2 * NT], F32, tag="hp", name="hp")
                    sp = spt[0:DIM_P, 0:NT]
                    for kc in range(4):
                        nc.tensor.matmul(
                            sp[:, :],
                            wo_sb[:, kc * DIM_P:(kc + 1) * DIM_P],
                            hs3[i][:, kc * NT:(kc + 1) * NT],
                            start=(kc == 0), stop=(kc == 3))
                    in1_q = (y_sb[g * TPG + i][0:16, :] if s == 1
                             else rhs1[i][0:16, :].bitcast(F32))
                    nc.vector.scalar_tensor_tensor(
                        out=q_sb[s][i][:, :],
                        in0=sp[:, :], scalar=bout_ap, in1=in1_q,
                        op0=ALU.add, op1=ALU.add)
                    if after_q is not None:
                        # urgent follow-up (e.g. th_{s+1} += c*q_s) directly
                        # behind this tile's q in the in-order DVE queue, so
                        # the next stage's L1 isn't stuck behind the other
                        # tiles' q ops
                        after_q(i)

            def step_body(g, t_n, dt_n, last):
                """One DOPRI5 step.  RK combinations are EAGER: as soon as
                q_j lands, every accumulator that needs it (th_{s'>j}, y) is
                updated, so the only DVE op between q_s and the next stage's
                L1 matmul is the single th_{s+1} += c*q_s term.  The update
                order (j ascending) is identical to the lazy chain, so the
                arithmetic is bit-identical."""
                # combination scalars k_j = bf_j * q_j, bf_j = beta factor at
                # the time k_j was evaluated — all compile-time floats.
                bf = [_beta_factor(t_n + C[j - 1] * dt_n) for j in range(1, 7)]

                def cs(coef, j):
                    return float(dt_n * coef * bf[j - 1])

                def th_upd(s_tgt, j, i):
                    first = (j == 1)
                    nc.vector.scalar_tensor_tensor(
                        out=th_sb[s_tgt][i][0:16, :],
                        in0=q_sb[j][i][:, :],
                        scalar=cs(A[s_tgt][j - 1], j),
                        in1=(y_sb[g * TPG + i][0:16, :] if first
                             else th_sb[s_tgt][i][0:16, :].bitcast(F32)),
                        op0=ALU.mult, op1=ALU.add)

                def y_upd(j, i):
                    if B[j - 1] == 0.0:
                        return
                    nc.vector.scalar_tensor_tensor(
                        out=y_sb[g * TPG + i][0:16, :],
                        in0=q_sb[j][i][:, :],
                        scalar=cs(B[j - 1], j),
                        in1=y_sb[g * TPG + i][0:16, :],
                        op0=ALU.mult, op1=ALU.add)

                # all six active-L1 lhsT tiles for this step, off-path
                for s in (2, 3, 4, 5, 6):
                    set_w1act(s, t_n + C[s - 1] * dt_n)
                # q1 terms: th_s inits (th_2 first — it gates stage 2's L1),
                # then y's k1 term
                for i in range(TPG):
                    th_upd(2, 1, i)
                for s_tgt in (3, 4, 5, 6):
                    for i in range(TPG):
                        th_upd(s_tgt, 1, i)
                for i in range(TPG):
                    y_upd(1, i)
                for s in (2, 3, 4, 5, 6):
                    # the q_s term of th_{s+1} gates the next stage's L1 —
                    # emit it directly after each tile's q op
                    hook = (lambda i, s=s: th_upd(s + 1, s, i)) if s < 6 else None
                    mlp_stage_all(s, g, slot=s, after_q=hook)
                    if s < 6:
                        for s_tgt in range(s + 2, 7):
                            for i in range(TPG):
                                th_upd(s_tgt, s, i)
                    for i in range(TPG):
                        y_upd(s, i)
                if last:
                    return
                # FSAL: k1 of the next step = drift(t+dt, y_new).  The L1
                # lhsT for t+dt is stage 6's (C6 == 1.0) — reuse slot 6.
                for i in range(TPG):
                    nc.vector.tensor_copy(yr_sb[i][:, :], y_sb[g * TPG + i][:, :])
                mlp_stage_all(1, g, slot=6)

            for g in range(n_groups):
                # k1 = drift(T1, y0) once per group
                set_w1act(1, grid[0])
                for i in range(TPG):
                    nc.vector.tensor_copy(yr_sb[i][:, :], y_sb[g * TPG + i][:, :])
                mlp_stage_all(1, g, slot=1)
                for n in range(n_steps):
                    step_body(g, grid[n], grid[n + 1] - grid[n],
                              last=(n == n_steps - 1))

    # ---- context 3: denormalize in place (rows 0:16 of ypack), then one
    #      strided output store (feature-major; host transposes) ----
    with tile.TileContext(nc):
        for gt in range(n_tiles):
            nc.vector.tensor_scalar(
                y_sb[gt][0:16, :], y_sb[gt][0:16, :],
                pstd_ap, pmean_ap,
                ALU.mult, ALU.add)
        nc.sync.dma_start(
            out=out_d.rearrange("(t p) n -> p t n", p=DIM_P),
            in_=ypack_sb[0:16, :].rearrange("p (t n) -> p t n", n=NT))

    _fix_sync_wait_overflow(nc, wsem)
    return nc


def unpack_out(outpack):
    """[n_tiles*16, NT] feature-major -> [n, 16] sample-major."""
    n_tiles = outpack.shape[0] // DIM_P
    return np.concatenate(
        [outpack[t * DIM_P:(t + 1) * DIM_P, :].T for t in range(n_tiles)], axis=0)


def kernel(**inputs) -> np.ndarray:
    host = prepare_host_inputs(**inputs)
    with_b23 = bool(np.any(host["b2"]) or np.any(host["b3"]))
    nc = build_program(with_b23=with_b23)

    base_map = {}
    if with_b23:
        b23 = np.zeros((128, 8), np.float32)
        b23[:, 0:4] = host["b2"].reshape(4, 128).T
        b23[:, 4:8] = host["b3"].reshape(4, 128).T
        base_map["b23pack"] = b23

    theta = host["theta"]
    in_maps = []
    for c in range(N_CORES):
        m = dict(base_map)
        m["megapack"] = pack_mega(host, theta[c * PER_CORE:(c + 1) * PER_CORE])
        in_maps.append(m)

    res = run_bass_kernel_spmd(nc, in_maps, list(range(N_CORES)))
    out = np.concatenate([unpack_out(res.results[c]["out"])
                          for c in range(N_CORES)], axis=0)
    return np.ascontiguousarray(out, np.float32)


if __name__ == "__main__":
    rng = np.random.default_rng(0)
    ins = {
        "x": rng.standard_normal(DIM_D).astype(np.float32),
        "init_theta": rng.standard_normal((N_SAMPLES, DIM_P)).astype(np.float32),
        "W1": rng.standard_normal((81, HID)).astype(np.float32) / 9.0,
        "b1": np.zeros(HID, np.float32),
        "W2": rng.standard_normal((HID, HID)).astype(np.float32) / 22.6,
        "b2": np.zeros(HID, np.float32),
        "W3": rng.standard_normal((HID, HID)).astype(np.float32) / 22.6,
        "b3": np.zeros(HID, np.float32),
        "Wout": rng.standard_normal((HID, DIM_P)).astype(np.float32) / 22.6,
        "bout": np.zeros(DIM_P, np.float32),
        "parameter_mean": rng.standard_normal(DIM_P).astype(np.float32),
        "parameter_std": np.ones(DIM_P, np.float32),
        "data_mean": rng.standard_normal(DIM_D).astype(np.float32),
        "data_std": np.ones(DIM_D, np.float32),
    }
    out = kernel(**ins)
    print(out.shape, out.dtype, np.abs(out).mean())
